# revision 1
# baseline (speedup 1.0000x reference)
"""EdgeDecoder Trainium2 kernel: out = 5*sigmoid(w2 . relu([z_u[row]; z_m[col]] @ W1.T + b1) + b2).

Strategy (8 NeuronCores, data-parallel over edges):
  1. Algebraic split: z @ W1.T = z_u[row] @ W1u.T + z_m[col] @ W1m.T, so precompute
     node tables A = |w2| * (z_u @ W1u.T + b1), B = |w2| * (z_m @ W1m.T) once on-device
     (PE matmuls), with the hidden dim permuted so positive-w2 units are contiguous.
  2. Per edge: two indirect-DMA row gathers (second accumulates via CCE add),
     relu as max(.,0) on positive block / min(.,0) on negative block (since
     w2[h]*relu(x) = relu(|w2|x) for w2>0 and = min(|w2|x, 0) for w2<0),
     segmented reduce over hidden, pos+neg, sigmoid(+b2) * 5.
Tables are replicated per core; each core processes a contiguous shard of edges.
"""
import sys
import numpy as np

sys.path.insert(0, '/opt/trn_rl_repo')

import concourse.bass as bass
import concourse.bacc as bacc
import concourse.mybir as mybir
import concourse.tile as tile
from concourse.bass_utils import run_bass_kernel_spmd

N_CORES = 8
P = 128
H = 128          # hidden
G = 32           # gather-loop cols per iteration (G*128 edges)
ZBODY = 1024     # precompute rows per loop body

_LAST_STATS = {}


def _build_nc(C, NA, NB, Hp, repeat=1, repeat_pre=None, repeat_gather=None):
    """C: edge cols per core (edges = 128*C). NA/NB: padded table rows. Hp: # pos-w2 units.
    repeat>1 re-runs the compute phases (identical results) for slope-based timing."""
    f32 = mybir.dt.float32
    i32 = mybir.dt.int32
    nc = bacc.Bacc(None, target_bir_lowering=False)

    zTu = nc.dram_tensor("zTu", [P, NA], f32, kind="ExternalInput")
    zTm = nc.dram_tensor("zTm", [P, NB], f32, kind="ExternalInput")
    w1ut = nc.dram_tensor("w1ut", [P, H], f32, kind="ExternalInput")
    w1mt = nc.dram_tensor("w1mt", [P, H], f32, kind="ExternalInput")
    b1rep = nc.dram_tensor("b1rep", [P, H], f32, kind="ExternalInput")
    b2rep = nc.dram_tensor("b2rep", [P, 1], f32, kind="ExternalInput")
    idxA = nc.dram_tensor("idxA", [P, C], i32, kind="ExternalInput")
    idxB = nc.dram_tensor("idxB", [P, C], i32, kind="ExternalInput")
    out_d = nc.dram_tensor("out", [P, C], f32, kind="ExternalOutput")

    tabA = nc.dram_tensor("tabA", [NA, H], f32)
    tabB = nc.dram_tensor("tabB", [NB, H], f32)
    # tile-linearized write view: table row (p*(N/128) + m) <-> partition p, col block m
    tabA_v = tabA[:].rearrange("(p m) d -> p (m d)", p=P)
    tabB_v = tabB[:].rearrange("(p m) d -> p (m d)", p=P)

    with tile.TileContext(nc) as tc:
        with (
            tc.tile_pool(name="const", bufs=1) as cpool,
            tc.tile_pool(name="work", bufs=3) as wpool,
            tc.tile_pool(name="psum", bufs=4, space="PSUM") as ppool,
        ):
            w1ut_t = cpool.tile([P, H], f32)
            w1mt_t = cpool.tile([P, H], f32)
            b1rep_t = cpool.tile([P, H], f32)
            b2rep_t = cpool.tile([P, 1], f32)
            idxA_t = cpool.tile([P, C], i32)
            idxB_t = cpool.tile([P, C], i32)
            logits = cpool.tile([P, C], f32)
            nc.sync.dma_start(out=w1ut_t[:], in_=w1ut[:])
            nc.sync.dma_start(out=w1mt_t[:], in_=w1mt[:])
            nc.sync.dma_start(out=b1rep_t[:], in_=b1rep[:])
            nc.sync.dma_start(out=b2rep_t[:], in_=b2rep[:])
            nc.sync.dma_start(out=idxA_t[:], in_=idxA[:])
            nc.sync.dma_start(out=idxB_t[:], in_=idxB[:])

            # ---- precompute tables ----
            for (zT, w1t, tab_v, npad, addb1) in (
                (zTu, w1ut_t, tabA_v, NA, True),
                (zTm, w1mt_t, tabB_v, NB, False),
            ) * (repeat_pre if repeat_pre is not None else repeat):
                with tc.For_i(0, npad, ZBODY) as iv:
                    zstage = wpool.tile([P, ZBODY], f32, tag="zstage")
                    nc.sync.dma_start(out=zstage[:], in_=zT[:, bass.ds(iv, ZBODY)])
                    astage = wpool.tile([P, ZBODY], f32, tag="astage")
                    for k in range(ZBODY // P):
                        ps = ppool.tile([P, H], f32, tag="ps")
                        nc.tensor.matmul(
                            out=ps[:],
                            lhsT=zstage[:, k * P:(k + 1) * P],
                            rhs=w1t[:],
                            start=True, stop=True,
                        )
                        sl = astage[:, k * H:(k + 1) * H]
                        if addb1:
                            nc.vector.tensor_add(out=sl, in0=ps[:], in1=b1rep_t[:])
                        else:
                            nc.scalar.copy(out=sl, in_=ps[:])
                    nc.sync.dma_start(out=tab_v[:, bass.ds(iv, ZBODY)], in_=astage[:])

            # ---- edge gather + MLP ----
            def gather_body(iv):
                rstage = wpool.tile([P, G], i32, tag="rstage")
                cstage = wpool.tile([P, G], i32, tag="cstage")
                nc.vector.tensor_copy(out=rstage[:], in_=idxA_t[:, bass.ds(iv, G)])
                nc.vector.tensor_copy(out=cstage[:], in_=idxB_t[:, bass.ds(iv, G)])
                ct = wpool.tile([P, G * H], f32, tag="ct")
                for j in range(G):
                    sl = ct[:, j * H:(j + 1) * H]
                    nc.gpsimd.indirect_dma_start(
                        out=sl, out_offset=None, in_=tabA[:],
                        in_offset=bass.IndirectOffsetOnAxis(ap=rstage[:, j:j + 1], axis=0),
                    )
                    nc.gpsimd.indirect_dma_start(
                        out=sl, out_offset=None, in_=tabB[:],
                        in_offset=bass.IndirectOffsetOnAxis(ap=cstage[:, j:j + 1], axis=0),
                        compute_op=mybir.AluOpType.add,
                    )
                cc = ct[:].rearrange("p (g h) -> p g h", h=H)
                if Hp > 0:
                    nc.vector.tensor_scalar_max(out=cc[:, :, 0:Hp], in0=cc[:, :, 0:Hp], scalar1=0.0)
                if Hp < H:
                    nc.vector.tensor_scalar_min(out=cc[:, :, Hp:H], in0=cc[:, :, Hp:H], scalar1=0.0)
                lsl = logits[:, bass.ds(iv, G)]
                if Hp == H or Hp == 0:
                    nc.vector.tensor_reduce(out=lsl, in_=cc[:, :, :], axis=mybir.AxisListType.X,
                                            op=mybir.AluOpType.add)
                else:
                    pos = wpool.tile([P, G], f32, tag="pos")
                    nc.vector.tensor_reduce(out=pos[:], in_=cc[:, :, 0:Hp],
                                            axis=mybir.AxisListType.X, op=mybir.AluOpType.add)
                    neg = wpool.tile([P, G], f32, tag="neg")
                    nc.vector.tensor_reduce(out=neg[:], in_=cc[:, :, Hp:H],
                                            axis=mybir.AxisListType.X, op=mybir.AluOpType.add)
                    nc.vector.tensor_add(out=lsl, in0=pos[:], in1=neg[:])

            for _rep in range(repeat_gather if repeat_gather is not None else repeat):
                with tc.For_i(0, C, G) as iv:
                    gather_body(iv)

            # ---- sigmoid tail ----
            sig = cpool.tile([P, C], f32)
            nc.scalar.activation(out=sig[:], in_=logits[:],
                                 func=mybir.ActivationFunctionType.Sigmoid,
                                 bias=b2rep_t[:, 0:1], scale=1.0)
            nc.scalar.mul(out=sig[:], in_=sig[:], mul=5.0)
            nc.sync.dma_start(out=out_d[:], in_=sig[:])
    nc.finalize()
    return nc


def _pad_cols(n, mult):
    return ((n + mult - 1) // mult) * mult


# ---------------------------------------------------------------------------
# v2: edges sorted by col; B-side expanded on PE via host-built one-hot S
# matrices (contiguous DMA) against B-chunks computed in-loop; A-side keeps
# per-row indirect gathers, accumulating onto the PE base via CCE add.
# ---------------------------------------------------------------------------

CPB = 2  # chunks per loop body


def _build_nc_v2(NA, NB, Hp, cap, repeat=1):
    f32 = mybir.dt.float32
    i32 = mybir.dt.int32
    MCH = NB // P               # number of 128-row B chunks
    SLOTC = MCH * cap           # slot columns ( = slots/128 ) per core
    nc = bacc.Bacc(None, target_bir_lowering=False)

    zTu = nc.dram_tensor("zTu", [P, NA], f32, kind="ExternalInput")
    zTm = nc.dram_tensor("zTm", [P, NB], f32, kind="ExternalInput")
    w1ut = nc.dram_tensor("w1ut", [P, H], f32, kind="ExternalInput")
    w1mt = nc.dram_tensor("w1mt", [P, H], f32, kind="ExternalInput")
    b1rep = nc.dram_tensor("b1rep", [P, H], f32, kind="ExternalInput")
    b2rep = nc.dram_tensor("b2rep", [P, 1], f32, kind="ExternalInput")
    sblob = nc.dram_tensor("sblob", [P, SLOTC * P], f32, kind="ExternalInput")
    idxA = nc.dram_tensor("idxA", [P, SLOTC], i32, kind="ExternalInput")
    out_d = nc.dram_tensor("out", [P, SLOTC], f32, kind="ExternalOutput")

    tabA = nc.dram_tensor("tabA", [NA, H], f32)
    tabA_v = tabA[:].rearrange("(p m) d -> p (m d)", p=P)
    SP = mybir.EngineType.SP

    with tile.TileContext(nc) as tc:
        with (
            tc.tile_pool(name="const", bufs=1) as cpool,
            tc.tile_pool(name="work", bufs=3) as wpool,
            tc.tile_pool(name="psum", bufs=2, space="PSUM") as ppool,
        ):
            w1ut_t = cpool.tile([P, H], f32)
            w1mt_t = cpool.tile([P, H], f32)
            b1rep_t = cpool.tile([P, H], f32)
            b2rep_t = cpool.tile([P, 1], f32)
            nc.sync.dma_start(out=w1ut_t[:], in_=w1ut[:])
            nc.sync.dma_start(out=w1mt_t[:], in_=w1mt[:])
            nc.sync.dma_start(out=b1rep_t[:], in_=b1rep[:])
            nc.sync.dma_start(out=b2rep_t[:], in_=b2rep[:])

            # ---- precompute A table (same as v1) ----
            for _r in range(repeat):
                with tc.For_i(0, NA, ZBODY) as iv:
                    zstage = wpool.tile([P, ZBODY], f32, tag="zstage")
                    nc.sync.dma_start(out=zstage[:], in_=zTu[:, bass.ds(iv, ZBODY)])
                    astage = wpool.tile([P, ZBODY], f32, tag="astage")
                    for k in range(ZBODY // P):
                        ps = ppool.tile([P, H], f32, tag="ps")
                        nc.tensor.matmul(out=ps[:], lhsT=zstage[:, k * P:(k + 1) * P],
                                         rhs=w1ut_t[:], start=True, stop=True)
                        nc.vector.tensor_add(out=astage[:, k * H:(k + 1) * H],
                                             in0=ps[:], in1=b1rep_t[:])
                    nc.sync.dma_start(out=tabA_v[:, bass.ds(iv, ZBODY)], in_=astage[:])

            # ---- main loop over B chunks; iv = chunk index m ----
            for _r in range(repeat):
                with tc.For_i(0, MCH, CPB) as iv:
                    # single (unchained) SP mults from iv for every dynamic offset
                    r_z = nc.alloc_registers(engines=[SP])
                    nc.regs_alu(r_z, iv, P, mybir.AluOpType.mult)
                    sv_z = nc.snap(r_z, donate=True)          # m*128  (zTm cols)
                    r_s = nc.alloc_registers(engines=[SP])
                    nc.regs_alu(r_s, iv, cap * P, mybir.AluOpType.mult)
                    sv_s = nc.snap(r_s, donate=True)          # m*cap*128 (sblob cols)
                    r_i = nc.alloc_registers(engines=[SP])
                    nc.regs_alu(r_i, iv, cap, mybir.AluOpType.mult)
                    sv_i = nc.snap(r_i, donate=True)          # m*cap (idx cols)
                    r_o = nc.alloc_registers(engines=[SP])
                    nc.regs_alu(r_o, iv, cap, mybir.AluOpType.mult)
                    sv_o = nc.snap(r_o, donate=True)          # m*cap (out cols)

                    zmst = wpool.tile([P, CPB * P], f32, tag="zmst")
                    nc.sync.dma_start(out=zmst[:], in_=zTm[:, bass.ds(sv_z, CPB * P)])
                    sstage = wpool.tile([P, CPB * cap * P], f32, tag="sstage")
                    nc.sync.dma_start(out=sstage[:], in_=sblob[:, bass.ds(sv_s, CPB * cap * P)])
                    ixst = wpool.tile([P, CPB * cap], i32, tag="ixst")
                    nc.sync.dma_start(out=ixst[:], in_=idxA[:, bass.ds(sv_i, CPB * cap)])

                    ct = wpool.tile([P, CPB * cap * H], f32, tag="ct")
                    for cc_i in range(CPB):
                        bps = ppool.tile([P, H], f32, tag="bps")
                        nc.tensor.matmul(out=bps[:], lhsT=zmst[:, cc_i * P:(cc_i + 1) * P],
                                         rhs=w1mt_t[:], start=True, stop=True)
                        bch = wpool.tile([P, H], f32, tag="bch")
                        nc.vector.tensor_copy(out=bch[:], in_=bps[:])
                        half = (cap + 1) // 2
                        for grp in range(2):
                            t0, t1 = grp * half, min((grp + 1) * half, cap)
                            if t0 >= t1:
                                continue
                            nts = t1 - t0
                            cps = ppool.tile([P, half * H], f32, tag="cps")
                            for t in range(t0, t1):
                                so = (cc_i * cap + t) * P
                                nc.tensor.matmul(
                                    out=cps[:, (t - t0) * H:(t - t0 + 1) * H],
                                    lhsT=sstage[:, so:so + P],
                                    rhs=bch[:], start=True, stop=True)
                            co = (cc_i * cap + t0) * H
                            nc.scalar.copy(out=ct[:, co:co + nts * H], in_=cps[:, :nts * H])
                        import os as _os
                        if _os.environ.get("EDGE_V2_NO_GATHER") == "1":
                            pass
                        elif _os.environ.get("EDGE_V2_PLAIN_GATHER") == "1":
                            ct2 = wpool.tile([P, CPB * cap * H], f32, tag="ct2")
                            for t in range(cap):
                                co = (cc_i * cap + t) * H
                                nc.gpsimd.indirect_dma_start(
                                    out=ct2[:, co:co + H], out_offset=None, in_=tabA[:],
                                    in_offset=bass.IndirectOffsetOnAxis(
                                        ap=ixst[:, cc_i * cap + t:cc_i * cap + t + 1], axis=0),
                                )
                            nc.vector.tensor_add(
                                out=ct[:, cc_i * cap * H:(cc_i + 1) * cap * H],
                                in0=ct[:, cc_i * cap * H:(cc_i + 1) * cap * H],
                                in1=ct2[:, cc_i * cap * H:(cc_i + 1) * cap * H])
                        else:
                            for t in range(cap):
                                co = (cc_i * cap + t) * H
                                nc.gpsimd.indirect_dma_start(
                                    out=ct[:, co:co + H], out_offset=None, in_=tabA[:],
                                    in_offset=bass.IndirectOffsetOnAxis(
                                        ap=ixst[:, cc_i * cap + t:cc_i * cap + t + 1], axis=0),
                                    compute_op=mybir.AluOpType.add,
                                )
                    ccv = ct[:].rearrange("p (g h) -> p g h", h=H)
                    if Hp > 0:
                        nc.vector.tensor_scalar_max(out=ccv[:, :, 0:Hp], in0=ccv[:, :, 0:Hp], scalar1=0.0)
                    if Hp < H:
                        nc.vector.tensor_scalar_min(out=ccv[:, :, Hp:H], in0=ccv[:, :, Hp:H], scalar1=0.0)
                    lg = wpool.tile([P, CPB * cap], f32, tag="lg")
                    if Hp == H or Hp == 0:
                        nc.vector.tensor_reduce(out=lg[:], in_=ccv[:, :, :],
                                                axis=mybir.AxisListType.X, op=mybir.AluOpType.add)
                    else:
                        pos = wpool.tile([P, CPB * cap], f32, tag="pos")
                        nc.vector.tensor_reduce(out=pos[:], in_=ccv[:, :, 0:Hp],
                                                axis=mybir.AxisListType.X, op=mybir.AluOpType.add)
                        neg = wpool.tile([P, CPB * cap], f32, tag="neg")
                        nc.vector.tensor_reduce(out=neg[:], in_=ccv[:, :, Hp:H],
                                                axis=mybir.AxisListType.X, op=mybir.AluOpType.add)
                        nc.vector.tensor_add(out=lg[:], in0=pos[:], in1=neg[:])
                    sg = wpool.tile([P, CPB * cap], f32, tag="sg")
                    nc.scalar.activation(out=sg[:], in_=lg[:],
                                         func=mybir.ActivationFunctionType.Sigmoid,
                                         bias=b2rep_t[:, 0:1], scale=1.0)
                    nc.scalar.mul(out=sg[:], in_=sg[:], mul=5.0)
                    nc.sync.dma_start(out=out_d[:, bass.ds(sv_o, CPB * cap)], in_=sg[:])
    nc.finalize()
    return nc


def _prepare(z_user, z_movie, edge_index, W1, b1, W2, b2, n_cores=N_CORES):
    z_user = np.asarray(z_user, dtype=np.float32)
    z_movie = np.asarray(z_movie, dtype=np.float32)
    edge_index = np.asarray(edge_index)
    W1 = np.asarray(W1, dtype=np.float32)
    b1 = np.asarray(b1, dtype=np.float32)
    W2 = np.asarray(W2, dtype=np.float32)
    b2 = np.asarray(b2, dtype=np.float32)

    E = edge_index.shape[1]
    rows = edge_index[0].astype(np.int64)
    cols = edge_index[1].astype(np.int64)

    NAr = int(rows.max()) + 1 if E else 1          # referenced user rows
    NBr = z_movie.shape[0]
    NA = _pad_cols(max(NAr, ZBODY), ZBODY)
    NB = _pad_cols(max(NBr, ZBODY), ZBODY)

    # hidden permutation: positive-w2 units first; fold signed w2 and b1 into tables
    w2 = W2.reshape(-1)
    perm = np.argsort(w2 < 0, kind="stable")       # stable: positives (False) first
    Hp = int((w2 >= 0).sum())
    W1p = W1[perm]                                  # [H, 2H]
    b1p = b1[perm]
    scale = w2[perm]  # signed: w2*relu(x) = max0(w2*x) for w2>0, min0(w2*x) for w2<0
    w1ut = np.ascontiguousarray((W1p[:, :H] * scale[:, None]).T)   # [in, h]
    w1mt = np.ascontiguousarray((W1p[:, H:] * scale[:, None]).T)
    b1rep = np.tile(b1p * scale, (P, 1)).astype(np.float32)
    b2rep = np.full((P, 1), float(b2.reshape(-1)[0]), dtype=np.float32)

    # transposed, padded node features
    zTu = np.zeros((P, NA), dtype=np.float32)
    zTu[:, :NAr] = z_user[:NAr].T
    zTm = np.zeros((P, NB), dtype=np.float32)
    zTm[:, :NBr] = z_movie.T

    # tile-linearized table row index: u -> (u%128)*(N/128) + u//128
    mA, mB = NA // P, NB // P
    idxA_full = ((rows % P) * mA + rows // P).astype(np.int32)
    idxB_full = ((cols % P) * mB + cols // P).astype(np.int32)

    # shard edges: per core 128*C edges, C divisible by G
    C = _pad_cols(-(-E // (n_cores * P)), G)
    Epc = P * C
    Etot = n_cores * Epc
    idxA_pad = np.zeros(Etot, dtype=np.int32)
    idxA_pad[:E] = idxA_full
    idxB_pad = np.zeros(Etot, dtype=np.int32)
    idxB_pad[:E] = idxB_full

    in_maps = []
    for c in range(n_cores):
        sl = slice(c * Epc, (c + 1) * Epc)
        in_maps.append({
            "zTu": zTu, "zTm": zTm, "w1ut": w1ut, "w1mt": w1mt,
            "b1rep": b1rep, "b2rep": b2rep,
            "idxA": idxA_pad[sl].reshape(P, C),
            "idxB": idxB_pad[sl].reshape(P, C),
        })
    return in_maps, dict(C=C, NA=NA, NB=NB, Hp=Hp, E=E)


def _prepare_v2(z_user, z_movie, edge_index, W1, b1, W2, b2, n_cores=N_CORES):
    z_user = np.asarray(z_user, dtype=np.float32)
    z_movie = np.asarray(z_movie, dtype=np.float32)
    edge_index = np.asarray(edge_index)
    W1 = np.asarray(W1, dtype=np.float32)
    b1 = np.asarray(b1, dtype=np.float32)
    W2 = np.asarray(W2, dtype=np.float32)
    b2 = np.asarray(b2, dtype=np.float32)

    E = edge_index.shape[1]
    rows = edge_index[0].astype(np.int64)
    cols = edge_index[1].astype(np.int64)
    NAr = int(rows.max()) + 1 if E else 1
    NBr = z_movie.shape[0]
    NA = _pad_cols(max(NAr, ZBODY), ZBODY)
    NB = _pad_cols(max(NBr, ZBODY), ZBODY)
    MCH = NB // P
    assert MCH % CPB == 0

    w2 = W2.reshape(-1)
    perm = np.argsort(w2 < 0, kind="stable")
    Hp = int((w2 >= 0).sum())
    W1p = W1[perm]
    b1p = b1[perm]
    scale = w2[perm]
    w1ut = np.ascontiguousarray((W1p[:, :H] * scale[:, None]).T)
    w1mt = np.ascontiguousarray((W1p[:, H:] * scale[:, None]).T)
    b1rep = np.tile(b1p * scale, (P, 1)).astype(np.float32)
    b2rep = np.full((P, 1), float(b2.reshape(-1)[0]), dtype=np.float32)

    zTu = np.zeros((P, NA), dtype=np.float32)
    zTu[:, :NAr] = z_user[:NAr].T
    zTm = np.zeros((P, NB), dtype=np.float32)
    zTm[:, :NBr] = z_movie.T
    mA = NA // P

    # shard edges contiguously, then per-core sort by col
    Epc = -(-E // n_cores)
    core_data = []
    cap = 1
    for c in range(n_cores):
        sl = slice(c * Epc, min((c + 1) * Epc, E))
        rc, cc = rows[sl], cols[sl]
        order = np.argsort(cc, kind="stable")
        rs, cs = rc[order], cc[order]
        cnt = np.bincount(cs // P, minlength=MCH)
        cap = max(cap, int(-(-cnt.max() // P)))
        core_data.append((sl, order, rs, cs, cnt))

    SLOTC = MCH * cap
    in_maps, backmaps = [], []
    for (sl, order, rs, cs, cnt) in core_data:
        m_e = cs // P
        lu_e = (cs % P).astype(np.int64)
        start = np.zeros(MCH + 1, dtype=np.int64)
        np.cumsum(cnt, out=start[1:])
        j_e = np.arange(len(cs)) - start[m_e]
        t_e = j_e // P
        p_e = j_e % P
        slotcol = m_e * cap + t_e
        idxA_blob = np.zeros((P, SLOTC), dtype=np.int32)
        idxA_blob[p_e, slotcol] = ((rs % P) * mA + rs // P).astype(np.int32)
        s_blob = np.zeros((P, SLOTC * P), dtype=np.float32)
        s_blob[lu_e, slotcol * P + p_e] = 1.0
        in_maps.append({
            "zTu": zTu, "zTm": zTm, "w1ut": w1ut, "w1mt": w1mt,
            "b1rep": b1rep, "b2rep": b2rep,
            "sblob": s_blob, "idxA": idxA_blob,
        })
        backmaps.append((sl, order, p_e, slotcol))
    return in_maps, dict(NA=NA, NB=NB, Hp=Hp, cap=cap, E=E,
                         SLOTC=SLOTC, backmaps=backmaps)


def _unpack_v2(res, meta):
    out = np.empty(meta["E"], dtype=np.float32)
    for c, (sl, order, p_e, slotcol) in enumerate(meta["backmaps"]):
        vals = res.results[c]["out"]               # [P, SLOTC]
        sorted_vals = vals[p_e, slotcol]
        seg = np.empty(len(order), dtype=np.float32)
        seg[order] = sorted_vals
        out[sl] = seg
    return out


def kernel(z_user, z_movie, edge_index, W1, b1, W2, b2):
    import os
    use_v2 = os.environ.get("EDGE_KERNEL_V2") == "1"
    if use_v2:
        try:
            in_maps, meta = _prepare_v2(z_user, z_movie, edge_index, W1, b1, W2, b2)
            nc = _build_nc_v2(meta["NA"], meta["NB"], meta["Hp"], meta["cap"])
            res = run_bass_kernel_spmd(nc, in_maps, core_ids=list(range(N_CORES)))
            out = _unpack_v2(res, meta)
            _LAST_STATS.update(exec_time_ns=res.exec_time_ns, nc=nc,
                               in_maps=in_maps, meta=meta, version="v2")
            return out
        except Exception as e:
            print(f"[kernel] v2 path failed ({type(e).__name__}: {e}); falling back to v1",
                  file=sys.stderr)
    in_maps, meta = _prepare(z_user, z_movie, edge_index, W1, b1, W2, b2)
    nc = _build_nc(meta["C"], meta["NA"], meta["NB"], meta["Hp"])
    res = run_bass_kernel_spmd(nc, in_maps, core_ids=list(range(N_CORES)))
    out = np.concatenate([res.results[c]["out"].reshape(-1) for c in range(N_CORES)])
    _LAST_STATS.update(exec_time_ns=res.exec_time_ns, nc=nc,
                       in_maps=in_maps, meta=meta, version="v1")
    return out[:meta["E"]].astype(np.float32)



# revision 6
# speedup vs baseline: 1.1883x; 1.1883x over previous
"""EdgeDecoder Trainium2 kernel: out = 5*sigmoid(w2 . relu([z_u[row]; z_m[col]] @ W1.T + b1) + b2).

Strategy (8 NeuronCores, data-parallel over edges):
  1. Algebraic split: z @ W1.T = z_u[row] @ W1u.T + z_m[col] @ W1m.T, so precompute
     node tables A = |w2| * (z_u @ W1u.T + b1), B = |w2| * (z_m @ W1m.T) once on-device
     (PE matmuls, bf16), with the hidden dim permuted so positive-w2 units are contiguous.
  2. Per edge: two single-column (128-row) indirect-DMA gathers into separate
     tiles -- independent DMAs with no RMW pairing keep the in-order Pool queue
     streaming descriptor generation. Then a batched DVE add, one contiguous
     relu on ACT (|w2| scaling makes relu uniform: w2*relu(x) =
     sign(w2) * relu(|w2| x)), segmented reduce over the pos/neg hidden blocks
     on DVE, pos - neg, sigmoid(+b2) * 5.
Tables are replicated per core; each core processes a contiguous shard of edges.
"""
import sys
import numpy as np

sys.path.insert(0, '/opt/trn_rl_repo')

import ml_dtypes
import concourse.bass as bass
import concourse.bacc as bacc
import concourse.mybir as mybir
import concourse.tile as tile
from concourse.bass_utils import run_bass_kernel_spmd

N_CORES = 8
P = 128
H = 128          # hidden
G = 32           # gather cols per block (one 128-row indirect DMA per col)
ZBODY = 1024     # precompute rows per loop body

BF16 = True      # table/z dtype

_LAST_STATS = {}


def _build_nc(C, NA, NB, Hp, repeat=1, repeat_pre=None, repeat_gather=None):
    """C: edge cols per core (edges = 128*C). NA/NB: padded table rows. Hp: # pos-w2 units.
    repeat>1 re-runs the compute phases (identical results) for slope-based timing."""
    f32 = mybir.dt.float32
    i32 = mybir.dt.int32
    tdt = mybir.dt.bfloat16 if BF16 else f32
    nc = bacc.Bacc(None, target_bir_lowering=False)

    zTu = nc.dram_tensor("zTu", [P, NA], tdt, kind="ExternalInput")
    zTm = nc.dram_tensor("zTm", [P, NB], tdt, kind="ExternalInput")
    w1ut = nc.dram_tensor("w1ut", [P, H], tdt, kind="ExternalInput")
    w1mt = nc.dram_tensor("w1mt", [P, H], tdt, kind="ExternalInput")
    b1rep = nc.dram_tensor("b1rep", [P, H], f32, kind="ExternalInput")
    b2rep = nc.dram_tensor("b2rep", [P, 1], f32, kind="ExternalInput")
    idxA = nc.dram_tensor("idxA", [P, C], i32, kind="ExternalInput")
    idxB = nc.dram_tensor("idxB", [P, C], i32, kind="ExternalInput")
    out_d = nc.dram_tensor("out", [P, C], f32, kind="ExternalOutput")

    tabA = nc.dram_tensor("tabA", [NA, H], tdt)
    tabB = nc.dram_tensor("tabB", [NB, H], tdt)
    # tile-linearized write view: table row (p*(N/128) + m) <-> partition p, col block m
    tabA_v = tabA[:].rearrange("(p m) d -> p (m d)", p=P)
    tabB_v = tabB[:].rearrange("(p m) d -> p (m d)", p=P)

    rp = repeat_pre if repeat_pre is not None else repeat
    rg = repeat_gather if repeat_gather is not None else repeat

    with tile.TileContext(nc) as tc:
        with (
            tc.tile_pool(name="const", bufs=1) as cpool,
            tc.tile_pool(name="work", bufs=3) as wpool,
            tc.tile_pool(name="psum", bufs=4, space="PSUM") as ppool,
        ):
            w1ut_t = cpool.tile([P, H], tdt)
            w1mt_t = cpool.tile([P, H], tdt)
            b1rep_t = cpool.tile([P, H], f32)
            b2rep_t = cpool.tile([P, 1], f32)
            zbias_t = cpool.tile([P, 1], f32)
            idxA_t = cpool.tile([P, C], i32)
            idxB_t = cpool.tile([P, C], i32)
            logits = cpool.tile([P, C], f32)
            nc.sync.dma_start(out=w1ut_t[:], in_=w1ut[:])
            nc.sync.dma_start(out=w1mt_t[:], in_=w1mt[:])
            nc.sync.dma_start(out=b1rep_t[:], in_=b1rep[:])
            nc.sync.dma_start(out=b2rep_t[:], in_=b2rep[:])
            nc.sync.dma_start(out=idxA_t[:], in_=idxA[:])
            nc.sync.dma_start(out=idxB_t[:], in_=idxB[:])
            nc.vector.memset(zbias_t[:], 0.0)

            # ---- precompute tables ----
            for (zT, w1t, tab_v, npad, addb1) in (
                (zTu, w1ut_t, tabA_v, NA, True),
                (zTm, w1mt_t, tabB_v, NB, False),
            ) * rp:
                with tc.For_i(0, npad, ZBODY) as iv:
                    zstage = wpool.tile([P, ZBODY], tdt, tag="zstage")
                    nc.sync.dma_start(out=zstage[:], in_=zT[:, bass.ds(iv, ZBODY)])
                    astage = wpool.tile([P, ZBODY], tdt, tag="astage")
                    for k in range(ZBODY // P):
                        ps = ppool.tile([P, H], f32, tag="ps")
                        nc.tensor.matmul(
                            out=ps[:],
                            lhsT=zstage[:, k * P:(k + 1) * P],
                            rhs=w1t[:],
                            start=True, stop=True,
                        )
                        sl = astage[:, k * H:(k + 1) * H]
                        if addb1:
                            nc.vector.tensor_add(out=sl, in0=ps[:], in1=b1rep_t[:])
                        else:
                            nc.scalar.copy(out=sl, in_=ps[:])
                    nc.sync.dma_start(out=tab_v[:, bass.ds(iv, ZBODY)], in_=astage[:])

            # ---- edge gather + MLP ----
            # Single-column indirect gathers (128 rows / instruction): the only
            # form with correct DMA-completion semaphore accounting on HW
            # (multi-column gathers release consumers after the first 128-row
            # chunk). A- and B-side go to separate tiles with NO CCE pairing:
            # a CCE-add gather RMW-waits at the head of the in-order Pool
            # queue and stalls all later descriptor generation.
            for _rep in range(rg):
                with tc.For_i(0, C, G) as iv:
                    rstage = wpool.tile([P, G], i32, tag="rstage")
                    cstage = wpool.tile([P, G], i32, tag="cstage")
                    nc.vector.tensor_copy(out=rstage[:], in_=idxA_t[:, bass.ds(iv, G)])
                    nc.vector.tensor_copy(out=cstage[:], in_=idxB_t[:, bass.ds(iv, G)])
                    ct = wpool.tile([P, G * H], tdt, tag="ct")
                    ct2 = wpool.tile([P, G * H], tdt, tag="ct2")
                    for j in range(G):
                        nc.gpsimd.indirect_dma_start(
                            out=ct[:, j * H:(j + 1) * H], out_offset=None, in_=tabA[:],
                            in_offset=bass.IndirectOffsetOnAxis(ap=rstage[:, j:j + 1], axis=0),
                        )
                    for j in range(G):
                        nc.gpsimd.indirect_dma_start(
                            out=ct2[:, j * H:(j + 1) * H], out_offset=None, in_=tabB[:],
                            in_offset=bass.IndirectOffsetOnAxis(ap=cstage[:, j:j + 1], axis=0),
                        )
                    nc.vector.tensor_add(out=ct[:], in0=ct[:], in1=ct2[:])
                    nc.scalar.activation(out=ct[:], in_=ct[:],
                                         func=mybir.ActivationFunctionType.Relu,
                                         bias=zbias_t[:, 0:1], scale=1.0)
                    cc = ct[:].rearrange("p (g h) -> p g h", h=H)
                    lsl = logits[:, bass.ds(iv, G)]
                    if Hp == H:
                        nc.vector.tensor_reduce(out=lsl, in_=cc[:, :, :],
                                                axis=mybir.AxisListType.X,
                                                op=mybir.AluOpType.add)
                    elif Hp == 0:
                        neg = wpool.tile([P, G], f32, tag="neg")
                        nc.vector.tensor_reduce(out=neg[:], in_=cc[:, :, :],
                                                axis=mybir.AxisListType.X,
                                                op=mybir.AluOpType.add)
                        nc.vector.tensor_scalar_mul(out=lsl, in0=neg[:], scalar1=-1.0)
                    else:
                        pos = wpool.tile([P, G], f32, tag="pos")
                        nc.vector.tensor_reduce(out=pos[:], in_=cc[:, :, 0:Hp],
                                                axis=mybir.AxisListType.X,
                                                op=mybir.AluOpType.add)
                        neg = wpool.tile([P, G], f32, tag="neg")
                        nc.vector.tensor_reduce(out=neg[:], in_=cc[:, :, Hp:H],
                                                axis=mybir.AxisListType.X,
                                                op=mybir.AluOpType.add)
                        nc.vector.tensor_sub(out=lsl, in0=pos[:], in1=neg[:])

            # ---- sigmoid tail ----
            sig = cpool.tile([P, C], f32)
            nc.scalar.activation(out=sig[:], in_=logits[:],
                                 func=mybir.ActivationFunctionType.Sigmoid,
                                 bias=b2rep_t[:, 0:1], scale=1.0)
            nc.scalar.mul(out=sig[:], in_=sig[:], mul=5.0)
            nc.sync.dma_start(out=out_d[:], in_=sig[:])
    nc.finalize()
    return nc


def _pad_cols(n, mult):
    return ((n + mult - 1) // mult) * mult


def _prepare(z_user, z_movie, edge_index, W1, b1, W2, b2, n_cores=N_CORES):
    z_user = np.asarray(z_user, dtype=np.float32)
    z_movie = np.asarray(z_movie, dtype=np.float32)
    edge_index = np.asarray(edge_index)
    W1 = np.asarray(W1, dtype=np.float32)
    b1 = np.asarray(b1, dtype=np.float32)
    W2 = np.asarray(W2, dtype=np.float32)
    b2 = np.asarray(b2, dtype=np.float32)
    tnp = ml_dtypes.bfloat16 if BF16 else np.float32

    E = edge_index.shape[1]
    rows = edge_index[0].astype(np.int64)
    cols = edge_index[1].astype(np.int64)

    NAr = int(rows.max()) + 1 if E else 1          # referenced user rows
    NBr = z_movie.shape[0]
    NA = _pad_cols(max(NAr, ZBODY), ZBODY)
    NB = _pad_cols(max(NBr, ZBODY), ZBODY)

    # hidden permutation: positive-w2 units first; fold |w2| and b1 into tables
    w2 = W2.reshape(-1)
    perm = np.argsort(w2 < 0, kind="stable")       # stable: positives (False) first
    Hp = int((w2 >= 0).sum())
    W1p = W1[perm]                                  # [H, 2H]
    b1p = b1[perm]
    scale = np.abs(w2[perm])  # w2*relu(x) = sign(w2) * relu(|w2| x)
    w1ut = np.ascontiguousarray((W1p[:, :H] * scale[:, None]).T).astype(tnp)  # [in, h]
    w1mt = np.ascontiguousarray((W1p[:, H:] * scale[:, None]).T).astype(tnp)
    b1rep = np.tile(b1p * scale, (P, 1)).astype(np.float32)
    b2rep = np.full((P, 1), float(b2.reshape(-1)[0]), dtype=np.float32)

    # transposed, padded node features
    zTu = np.zeros((P, NA), dtype=tnp)
    zTu[:, :NAr] = z_user[:NAr].T.astype(tnp)
    zTm = np.zeros((P, NB), dtype=tnp)
    zTm[:, :NBr] = z_movie.T.astype(tnp)

    # tile-linearized table row index: u -> (u%128)*(N/128) + u//128
    mA, mB = NA // P, NB // P
    idxA_full = ((rows % P) * mA + rows // P).astype(np.int32)
    idxB_full = ((cols % P) * mB + cols // P).astype(np.int32)

    # shard edges: per core 128*C edges, C divisible by G
    C = _pad_cols(-(-E // (n_cores * P)), G)
    Epc = P * C
    Etot = n_cores * Epc
    idxA_pad = np.zeros(Etot, dtype=np.int32)
    idxA_pad[:E] = idxA_full
    idxB_pad = np.zeros(Etot, dtype=np.int32)
    idxB_pad[:E] = idxB_full

    in_maps = []
    for c in range(n_cores):
        sl = slice(c * Epc, (c + 1) * Epc)
        in_maps.append({
            "zTu": zTu, "zTm": zTm, "w1ut": w1ut, "w1mt": w1mt,
            "b1rep": b1rep, "b2rep": b2rep,
            "idxA": idxA_pad[sl].reshape(P, C),
            "idxB": idxB_pad[sl].reshape(P, C),
        })
    return in_maps, dict(C=C, NA=NA, NB=NB, Hp=Hp, E=E)


def kernel(z_user, z_movie, edge_index, W1, b1, W2, b2):
    in_maps, meta = _prepare(z_user, z_movie, edge_index, W1, b1, W2, b2)
    nc = _build_nc(meta["C"], meta["NA"], meta["NB"], meta["Hp"])
    res = run_bass_kernel_spmd(nc, in_maps, core_ids=list(range(N_CORES)))
    out = np.concatenate([res.results[c]["out"].reshape(-1) for c in range(N_CORES)])
    _LAST_STATS.update(exec_time_ns=res.exec_time_ns, nc=nc,
                       in_maps=in_maps, meta=meta, version="v3")
    return out[:meta["E"]].astype(np.float32)


# revision 7
# speedup vs baseline: 1.2701x; 1.0689x over previous
"""EdgeDecoder Trainium2 kernel: out = 5*sigmoid(w2 . relu([z_u[row]; z_m[col]] @ W1.T + b1) + b2).

Strategy (8 NeuronCores, data-parallel over edges):
  1. Algebraic split: z @ W1.T = z_u[row] @ W1u.T + z_m[col] @ W1m.T, so precompute
     node tables A = |w2| * (z_u @ W1u.T + b1), B = |w2| * (z_m @ W1m.T) once on-device
     (PE matmuls, bf16), with the hidden dim permuted so positive-w2 units are contiguous.
  2. Per edge: two single-column (128-row) indirect-DMA gathers into separate
     tiles -- independent DMAs with no RMW pairing keep the in-order Pool queue
     streaming descriptor generation. Then a batched DVE add, one contiguous
     relu on ACT (|w2| scaling makes relu uniform: w2*relu(x) =
     sign(w2) * relu(|w2| x)), segmented reduce over the pos/neg hidden blocks
     on DVE, pos - neg, sigmoid(+b2) * 5.
Tables are replicated per core; each core processes a contiguous shard of edges.
"""
import sys
import numpy as np

sys.path.insert(0, '/opt/trn_rl_repo')

import ml_dtypes
import concourse.bass as bass
import concourse.bacc as bacc
import concourse.mybir as mybir
import concourse.tile as tile
from concourse.bass_utils import run_bass_kernel_spmd

N_CORES = 8
P = 128
H = 128          # hidden
G = 64           # gather cols per block (one 128-row indirect DMA per col)
ZBODY = 1024     # precompute rows per loop body

BF16 = True      # table/z dtype

_LAST_STATS = {}


def _build_nc(C, NA, NB, Hp, repeat=1, repeat_pre=None, repeat_gather=None):
    """C: edge cols per core (edges = 128*C). NA/NB: padded table rows. Hp: # pos-w2 units.
    repeat>1 re-runs the compute phases (identical results) for slope-based timing."""
    f32 = mybir.dt.float32
    i32 = mybir.dt.int32
    tdt = mybir.dt.bfloat16 if BF16 else f32
    nc = bacc.Bacc(None, target_bir_lowering=False)

    zTu = nc.dram_tensor("zTu", [P, NA], tdt, kind="ExternalInput")
    zTm = nc.dram_tensor("zTm", [P, NB], tdt, kind="ExternalInput")
    w1ut = nc.dram_tensor("w1ut", [P, H], tdt, kind="ExternalInput")
    w1mt = nc.dram_tensor("w1mt", [P, H], tdt, kind="ExternalInput")
    b1rep = nc.dram_tensor("b1rep", [P, H], f32, kind="ExternalInput")
    b2rep = nc.dram_tensor("b2rep", [P, 1], f32, kind="ExternalInput")
    idxA = nc.dram_tensor("idxA", [P, C], i32, kind="ExternalInput")
    idxB = nc.dram_tensor("idxB", [P, C], i32, kind="ExternalInput")
    out_d = nc.dram_tensor("out", [P, C], f32, kind="ExternalOutput")

    tabA = nc.dram_tensor("tabA", [NA, H], tdt)
    tabB = nc.dram_tensor("tabB", [NB, H], tdt)
    # tile-linearized write view: table row (p*(N/128) + m) <-> partition p, col block m
    tabA_v = tabA[:].rearrange("(p m) d -> p (m d)", p=P)
    tabB_v = tabB[:].rearrange("(p m) d -> p (m d)", p=P)

    rp = repeat_pre if repeat_pre is not None else repeat
    rg = repeat_gather if repeat_gather is not None else repeat

    with tile.TileContext(nc) as tc:
        with (
            tc.tile_pool(name="const", bufs=1) as cpool,
            tc.tile_pool(name="work", bufs=4) as wpool,
            tc.tile_pool(name="psum", bufs=4, space="PSUM") as ppool,
        ):
            w1ut_t = cpool.tile([P, H], tdt)
            w1mt_t = cpool.tile([P, H], tdt)
            b1rep_t = cpool.tile([P, H], f32)
            b2rep_t = cpool.tile([P, 1], f32)
            zbias_t = cpool.tile([P, 1], f32)
            idxA_t = cpool.tile([P, C], i32)
            idxB_t = cpool.tile([P, C], i32)
            logits = cpool.tile([P, C], f32)
            nc.sync.dma_start(out=w1ut_t[:], in_=w1ut[:])
            nc.sync.dma_start(out=w1mt_t[:], in_=w1mt[:])
            nc.sync.dma_start(out=b1rep_t[:], in_=b1rep[:])
            nc.sync.dma_start(out=b2rep_t[:], in_=b2rep[:])
            nc.sync.dma_start(out=idxA_t[:], in_=idxA[:])
            nc.sync.dma_start(out=idxB_t[:], in_=idxB[:])
            nc.vector.memset(zbias_t[:], 0.0)

            # ---- precompute tables ----
            for (zT, w1t, tab_v, npad, addb1) in (
                (zTu, w1ut_t, tabA_v, NA, True),
                (zTm, w1mt_t, tabB_v, NB, False),
            ) * rp:
                with tc.For_i(0, npad, ZBODY) as iv:
                    zstage = wpool.tile([P, ZBODY], tdt, tag="zstage")
                    nc.sync.dma_start(out=zstage[:], in_=zT[:, bass.ds(iv, ZBODY)])
                    astage = wpool.tile([P, ZBODY], tdt, tag="astage")
                    for k in range(ZBODY // P):
                        ps = ppool.tile([P, H], f32, tag="ps")
                        nc.tensor.matmul(
                            out=ps[:],
                            lhsT=zstage[:, k * P:(k + 1) * P],
                            rhs=w1t[:],
                            start=True, stop=True,
                        )
                        sl = astage[:, k * H:(k + 1) * H]
                        if addb1:
                            nc.vector.tensor_add(out=sl, in0=ps[:], in1=b1rep_t[:])
                        else:
                            nc.scalar.copy(out=sl, in_=ps[:])
                    nc.sync.dma_start(out=tab_v[:, bass.ds(iv, ZBODY)], in_=astage[:])

            # ---- edge gather + MLP ----
            # Single-column indirect gathers (128 rows / instruction): the only
            # form with correct DMA-completion semaphore accounting on HW
            # (multi-column gathers release consumers after the first 128-row
            # chunk). A- and B-side go to separate tiles with NO CCE pairing:
            # a CCE-add gather RMW-waits at the head of the in-order Pool
            # queue and stalls all later descriptor generation.
            for _rep in range(rg):
                with tc.For_i(0, C, G) as iv:
                    rstage = wpool.tile([P, G], i32, tag="rstage")
                    cstage = wpool.tile([P, G], i32, tag="cstage")
                    nc.vector.tensor_copy(out=rstage[:], in_=idxA_t[:, bass.ds(iv, G)])
                    nc.vector.tensor_copy(out=cstage[:], in_=idxB_t[:, bass.ds(iv, G)])
                    ct = wpool.tile([P, G * H], tdt, tag="ct")
                    ct2 = wpool.tile([P, G * H], tdt, tag="ct2")
                    for j in range(G):
                        nc.gpsimd.indirect_dma_start(
                            out=ct[:, j * H:(j + 1) * H], out_offset=None, in_=tabA[:],
                            in_offset=bass.IndirectOffsetOnAxis(ap=rstage[:, j:j + 1], axis=0),
                        )
                    for j in range(G):
                        nc.gpsimd.indirect_dma_start(
                            out=ct2[:, j * H:(j + 1) * H], out_offset=None, in_=tabB[:],
                            in_offset=bass.IndirectOffsetOnAxis(ap=cstage[:, j:j + 1], axis=0),
                        )
                    nc.vector.tensor_add(out=ct[:], in0=ct[:], in1=ct2[:])
                    nc.scalar.activation(out=ct[:], in_=ct[:],
                                         func=mybir.ActivationFunctionType.Relu,
                                         bias=zbias_t[:, 0:1], scale=1.0)
                    cc = ct[:].rearrange("p (g h) -> p g h", h=H)
                    lsl = logits[:, bass.ds(iv, G)]
                    if Hp == H:
                        nc.vector.tensor_reduce(out=lsl, in_=cc[:, :, :],
                                                axis=mybir.AxisListType.X,
                                                op=mybir.AluOpType.add)
                    elif Hp == 0:
                        neg = wpool.tile([P, G], f32, tag="neg")
                        nc.vector.tensor_reduce(out=neg[:], in_=cc[:, :, :],
                                                axis=mybir.AxisListType.X,
                                                op=mybir.AluOpType.add)
                        nc.vector.tensor_scalar_mul(out=lsl, in0=neg[:], scalar1=-1.0)
                    else:
                        pos = wpool.tile([P, G], f32, tag="pos")
                        nc.vector.tensor_reduce(out=pos[:], in_=cc[:, :, 0:Hp],
                                                axis=mybir.AxisListType.X,
                                                op=mybir.AluOpType.add)
                        neg = wpool.tile([P, G], f32, tag="neg")
                        nc.vector.tensor_reduce(out=neg[:], in_=cc[:, :, Hp:H],
                                                axis=mybir.AxisListType.X,
                                                op=mybir.AluOpType.add)
                        nc.vector.tensor_sub(out=lsl, in0=pos[:], in1=neg[:])

            # ---- sigmoid tail ----
            sig = cpool.tile([P, C], f32)
            nc.scalar.activation(out=sig[:], in_=logits[:],
                                 func=mybir.ActivationFunctionType.Sigmoid,
                                 bias=b2rep_t[:, 0:1], scale=1.0)
            nc.scalar.mul(out=sig[:], in_=sig[:], mul=5.0)
            nc.sync.dma_start(out=out_d[:], in_=sig[:])
    nc.finalize()
    return nc


def _pad_cols(n, mult):
    return ((n + mult - 1) // mult) * mult


def _prepare(z_user, z_movie, edge_index, W1, b1, W2, b2, n_cores=N_CORES):
    z_user = np.asarray(z_user, dtype=np.float32)
    z_movie = np.asarray(z_movie, dtype=np.float32)
    edge_index = np.asarray(edge_index)
    W1 = np.asarray(W1, dtype=np.float32)
    b1 = np.asarray(b1, dtype=np.float32)
    W2 = np.asarray(W2, dtype=np.float32)
    b2 = np.asarray(b2, dtype=np.float32)
    tnp = ml_dtypes.bfloat16 if BF16 else np.float32

    E = edge_index.shape[1]
    rows = edge_index[0].astype(np.int64)
    cols = edge_index[1].astype(np.int64)

    NAr = int(rows.max()) + 1 if E else 1          # referenced user rows
    NBr = z_movie.shape[0]
    NA = _pad_cols(max(NAr, ZBODY), ZBODY)
    NB = _pad_cols(max(NBr, ZBODY), ZBODY)

    # hidden permutation: positive-w2 units first; fold |w2| and b1 into tables
    w2 = W2.reshape(-1)
    perm = np.argsort(w2 < 0, kind="stable")       # stable: positives (False) first
    Hp = int((w2 >= 0).sum())
    W1p = W1[perm]                                  # [H, 2H]
    b1p = b1[perm]
    scale = np.abs(w2[perm])  # w2*relu(x) = sign(w2) * relu(|w2| x)
    w1ut = np.ascontiguousarray((W1p[:, :H] * scale[:, None]).T).astype(tnp)  # [in, h]
    w1mt = np.ascontiguousarray((W1p[:, H:] * scale[:, None]).T).astype(tnp)
    b1rep = np.tile(b1p * scale, (P, 1)).astype(np.float32)
    b2rep = np.full((P, 1), float(b2.reshape(-1)[0]), dtype=np.float32)

    # transposed, padded node features
    zTu = np.zeros((P, NA), dtype=tnp)
    zTu[:, :NAr] = z_user[:NAr].T.astype(tnp)
    zTm = np.zeros((P, NB), dtype=tnp)
    zTm[:, :NBr] = z_movie.T.astype(tnp)

    # tile-linearized table row index: u -> (u%128)*(N/128) + u//128
    mA, mB = NA // P, NB // P
    idxA_full = ((rows % P) * mA + rows // P).astype(np.int32)
    idxB_full = ((cols % P) * mB + cols // P).astype(np.int32)

    # shard edges: per core 128*C edges, C divisible by G
    C = _pad_cols(-(-E // (n_cores * P)), G)
    Epc = P * C
    Etot = n_cores * Epc
    idxA_pad = np.zeros(Etot, dtype=np.int32)
    idxA_pad[:E] = idxA_full
    idxB_pad = np.zeros(Etot, dtype=np.int32)
    idxB_pad[:E] = idxB_full

    in_maps = []
    for c in range(n_cores):
        sl = slice(c * Epc, (c + 1) * Epc)
        in_maps.append({
            "zTu": zTu, "zTm": zTm, "w1ut": w1ut, "w1mt": w1mt,
            "b1rep": b1rep, "b2rep": b2rep,
            "idxA": idxA_pad[sl].reshape(P, C),
            "idxB": idxB_pad[sl].reshape(P, C),
        })
    return in_maps, dict(C=C, NA=NA, NB=NB, Hp=Hp, E=E)


def kernel(z_user, z_movie, edge_index, W1, b1, W2, b2):
    in_maps, meta = _prepare(z_user, z_movie, edge_index, W1, b1, W2, b2)
    nc = _build_nc(meta["C"], meta["NA"], meta["NB"], meta["Hp"])
    res = run_bass_kernel_spmd(nc, in_maps, core_ids=list(range(N_CORES)))
    out = np.concatenate([res.results[c]["out"].reshape(-1) for c in range(N_CORES)])
    _LAST_STATS.update(exec_time_ns=res.exec_time_ns, nc=nc,
                       in_maps=in_maps, meta=meta, version="v3")
    return out[:meta["E"]].astype(np.float32)


# revision 10
# speedup vs baseline: 1.3865x; 1.0916x over previous
"""EdgeDecoder Trainium2 kernel: out = 5*sigmoid(w2 . relu([z_u[row]; z_m[col]] @ W1.T + b1) + b2).

Strategy (8 NeuronCores, data-parallel over edges):
  1. Algebraic split: z @ W1.T = z_u[row] @ W1u.T + z_m[col] @ W1m.T, so precompute
     node tables A = |w2| * (z_u @ W1u.T + b1), B = |w2| * (z_m @ W1m.T) once on-device
     (PE matmuls, bf16), with the hidden dim permuted so positive-w2 units are contiguous.
  2. Per edge: two single-column (128-row) indirect-DMA gathers into separate
     tiles -- independent DMAs with no RMW pairing keep the in-order Pool queue
     streaming descriptor generation. Then a batched DVE add, one contiguous
     relu on ACT (|w2| scaling makes relu uniform: w2*relu(x) =
     sign(w2) * relu(|w2| x)), segmented reduce over the pos/neg hidden blocks
     on DVE, pos - neg, sigmoid(+b2) * 5.
Tables are replicated per core; each core processes a contiguous shard of edges.
"""
import sys
import numpy as np

sys.path.insert(0, '/opt/trn_rl_repo')

import ml_dtypes
import concourse.bass as bass
import concourse.bacc as bacc
import concourse.mybir as mybir
import concourse.tile as tile
from concourse.bass_utils import run_bass_kernel_spmd

N_CORES = 8
P = 128
H = 128          # hidden
G = 64           # gather cols per block (one 128-row indirect DMA per col)
ZBODY = 1024     # precompute rows per loop body

BF16 = True      # table/z dtype

_LAST_STATS = {}


def _build_nc(C, NA, NB, Hp, repeat=1, repeat_pre=None, repeat_gather=None):
    """C: edge cols per core (edges = 128*C). NA/NB: padded table rows. Hp: # pos-w2 units.
    repeat>1 re-runs the compute phases (identical results) for slope-based timing."""
    f32 = mybir.dt.float32
    i32 = mybir.dt.int32
    tdt = mybir.dt.bfloat16 if BF16 else f32
    nc = bacc.Bacc(None, target_bir_lowering=False)

    zTu = nc.dram_tensor("zTu", [P, NA], tdt, kind="ExternalInput")
    zTm = nc.dram_tensor("zTm", [P, NB], tdt, kind="ExternalInput")
    w1ut = nc.dram_tensor("w1ut", [P, H], tdt, kind="ExternalInput")
    w1mt = nc.dram_tensor("w1mt", [P, H], tdt, kind="ExternalInput")
    b1rep = nc.dram_tensor("b1rep", [P, H], f32, kind="ExternalInput")
    b2rep = nc.dram_tensor("b2rep", [P, 1], f32, kind="ExternalInput")
    idxA = nc.dram_tensor("idxA", [P, C], i32, kind="ExternalInput")
    idxB = nc.dram_tensor("idxB", [P, C], i32, kind="ExternalInput")
    out_d = nc.dram_tensor("out", [P, C], f32, kind="ExternalOutput")

    tabA = nc.dram_tensor("tabA", [NA, H], tdt)
    tabB = nc.dram_tensor("tabB", [NB, H], tdt)
    # tile-linearized write view: table row (p*(N/128) + m) <-> partition p, col block m
    tabA_v = tabA[:].rearrange("(p m) d -> p (m d)", p=P)
    tabB_v = tabB[:].rearrange("(p m) d -> p (m d)", p=P)

    rp = repeat_pre if repeat_pre is not None else repeat
    rg = repeat_gather if repeat_gather is not None else repeat

    with tile.TileContext(nc) as tc:
        with (
            tc.tile_pool(name="const", bufs=1) as cpool,
            tc.tile_pool(name="work", bufs=4) as wpool,
            tc.tile_pool(name="psum", bufs=4, space="PSUM") as ppool,
        ):
            w1ut_t = cpool.tile([P, H], tdt)
            w1mt_t = cpool.tile([P, H], tdt)
            b1rep_t = cpool.tile([P, H], f32)
            b2rep_t = cpool.tile([P, 1], f32)
            zbias_t = cpool.tile([P, 1], f32)
            idxA_t = cpool.tile([P, C], i32)
            idxB_t = cpool.tile([P, C], i32)
            logits = cpool.tile([P, C], f32)
            nc.sync.dma_start(out=w1ut_t[:], in_=w1ut[:])
            nc.sync.dma_start(out=w1mt_t[:], in_=w1mt[:])
            nc.sync.dma_start(out=b1rep_t[:], in_=b1rep[:])
            nc.sync.dma_start(out=b2rep_t[:], in_=b2rep[:])
            nc.sync.dma_start(out=idxA_t[:], in_=idxA[:])
            nc.sync.dma_start(out=idxB_t[:], in_=idxB[:])
            nc.vector.memset(zbias_t[:], 0.0)

            # ---- precompute tables ----
            for (zT, w1t, tab_v, npad, addb1) in (
                (zTu, w1ut_t, tabA_v, NA, True),
                (zTm, w1mt_t, tabB_v, NB, False),
            ) * rp:
                with tc.For_i(0, npad, ZBODY) as iv:
                    zstage = wpool.tile([P, ZBODY], tdt, tag="zstage")
                    nc.sync.dma_start(out=zstage[:], in_=zT[:, bass.ds(iv, ZBODY)])
                    astage = wpool.tile([P, ZBODY], tdt, tag="astage")
                    for k in range(ZBODY // P):
                        ps = ppool.tile([P, H], f32, tag="ps")
                        nc.tensor.matmul(
                            out=ps[:],
                            lhsT=zstage[:, k * P:(k + 1) * P],
                            rhs=w1t[:],
                            start=True, stop=True,
                        )
                        sl = astage[:, k * H:(k + 1) * H]
                        if addb1:
                            nc.vector.tensor_add(out=sl, in0=ps[:], in1=b1rep_t[:])
                        else:
                            nc.scalar.copy(out=sl, in_=ps[:])
                    nc.sync.dma_start(out=tab_v[:, bass.ds(iv, ZBODY)], in_=astage[:])

            # ---- edge gather + MLP ----
            # Single-column indirect gathers (128 rows / instruction): the only
            # form with correct DMA-completion semaphore accounting on HW
            # (multi-column gathers release consumers after the first 128-row
            # chunk). A- and B-side go to separate tiles with NO CCE pairing:
            # a CCE-add gather RMW-waits at the head of the in-order Pool
            # queue and stalls all later descriptor generation.
            for _rep in range(rg):
                with tc.For_i(0, C, G) as iv:
                    rstage = wpool.tile([P, G], i32, tag="rstage")
                    cstage = wpool.tile([P, G], i32, tag="cstage")
                    nc.vector.tensor_copy(out=rstage[:], in_=idxA_t[:, bass.ds(iv, G)])
                    nc.vector.tensor_copy(out=cstage[:], in_=idxB_t[:, bass.ds(iv, G)])
                    ct = wpool.tile([P, G * H], tdt, tag="ct")
                    ct2 = wpool.tile([P, G * H], tdt, tag="ct2")
                    for j in range(G):
                        nc.gpsimd.indirect_dma_start(
                            out=ct[:, j * H:(j + 1) * H], out_offset=None, in_=tabA[:],
                            in_offset=bass.IndirectOffsetOnAxis(ap=rstage[:, j:j + 1], axis=0),
                        )
                    for j in range(G):
                        nc.gpsimd.indirect_dma_start(
                            out=ct2[:, j * H:(j + 1) * H], out_offset=None, in_=tabB[:],
                            in_offset=bass.IndirectOffsetOnAxis(ap=cstage[:, j:j + 1], axis=0),
                        )
                    nc.vector.tensor_add(out=ct[:], in0=ct[:], in1=ct2[:])
                    nc.scalar.activation(out=ct[:], in_=ct[:],
                                         func=mybir.ActivationFunctionType.Relu,
                                         bias=zbias_t[:, 0:1], scale=1.0)
                    cc = ct[:].rearrange("p (g h) -> p g h", h=H)
                    lsl = logits[:, bass.ds(iv, G)]
                    if Hp == H:
                        nc.vector.tensor_reduce(out=lsl, in_=cc[:, :, :],
                                                axis=mybir.AxisListType.X,
                                                op=mybir.AluOpType.add)
                    elif Hp == 0:
                        neg = wpool.tile([P, G], f32, tag="neg")
                        nc.vector.tensor_reduce(out=neg[:], in_=cc[:, :, :],
                                                axis=mybir.AxisListType.X,
                                                op=mybir.AluOpType.add)
                        nc.vector.tensor_scalar_mul(out=lsl, in0=neg[:], scalar1=-1.0)
                    else:
                        pos = wpool.tile([P, G], f32, tag="pos")
                        nc.vector.tensor_reduce(out=pos[:], in_=cc[:, :, 0:Hp],
                                                axis=mybir.AxisListType.X,
                                                op=mybir.AluOpType.add)
                        neg = wpool.tile([P, G], f32, tag="neg")
                        nc.vector.tensor_reduce(out=neg[:], in_=cc[:, :, Hp:H],
                                                axis=mybir.AxisListType.X,
                                                op=mybir.AluOpType.add)
                        nc.vector.tensor_sub(out=lsl, in0=pos[:], in1=neg[:])

            # ---- sigmoid tail ----
            sig = cpool.tile([P, C], f32)
            nc.scalar.activation(out=sig[:], in_=logits[:],
                                 func=mybir.ActivationFunctionType.Sigmoid,
                                 bias=b2rep_t[:, 0:1], scale=1.0)
            nc.scalar.mul(out=sig[:], in_=sig[:], mul=5.0)
            nc.sync.dma_start(out=out_d[:], in_=sig[:])
    nc.finalize()
    return nc


def _pad_cols(n, mult):
    return ((n + mult - 1) // mult) * mult


def _prepare(z_user, z_movie, edge_index, W1, b1, W2, b2, n_cores=N_CORES):
    z_user = np.asarray(z_user, dtype=np.float32)
    z_movie = np.asarray(z_movie, dtype=np.float32)
    edge_index = np.asarray(edge_index)
    W1 = np.asarray(W1, dtype=np.float32)
    b1 = np.asarray(b1, dtype=np.float32)
    W2 = np.asarray(W2, dtype=np.float32)
    b2 = np.asarray(b2, dtype=np.float32)
    tnp = ml_dtypes.bfloat16 if BF16 else np.float32

    E = edge_index.shape[1]
    rows = edge_index[0].astype(np.int64)
    cols = edge_index[1].astype(np.int64)

    NAr = int(rows.max()) + 1 if E else 1          # referenced user rows
    NBr = z_movie.shape[0]
    NA = _pad_cols(max(NAr, ZBODY), ZBODY)
    NB = _pad_cols(max(NBr, ZBODY), ZBODY)

    # hidden permutation: positive-w2 units first; fold |w2| and b1 into tables
    w2 = W2.reshape(-1)
    perm = np.argsort(w2 < 0, kind="stable")       # stable: positives (False) first
    Hp = int((w2 >= 0).sum())
    W1p = W1[perm]                                  # [H, 2H]
    b1p = b1[perm]
    scale = np.abs(w2[perm])  # w2*relu(x) = sign(w2) * relu(|w2| x)
    w1ut = np.ascontiguousarray((W1p[:, :H] * scale[:, None]).T).astype(tnp)  # [in, h]
    w1mt = np.ascontiguousarray((W1p[:, H:] * scale[:, None]).T).astype(tnp)
    b1rep = np.tile(b1p * scale, (P, 1)).astype(np.float32)
    b2rep = np.full((P, 1), float(b2.reshape(-1)[0]), dtype=np.float32)

    # transposed, padded node features
    zTu = np.zeros((P, NA), dtype=tnp)
    zTu[:, :NAr] = z_user[:NAr].T.astype(tnp)
    zTm = np.zeros((P, NB), dtype=tnp)
    zTm[:, :NBr] = z_movie.T.astype(tnp)

    # tile-linearized table row index: u -> (u%128)*(N/128) + u//128
    mA, mB = NA // P, NB // P
    idxA_full = ((rows % P) * mA + rows // P).astype(np.int32)
    idxB_full = ((cols % P) * mB + cols // P).astype(np.int32)

    # shard edges: per core 128*C edges, C divisible by G
    C = _pad_cols(-(-E // (n_cores * P)), G)
    Epc = P * C
    Etot = n_cores * Epc
    idxA_pad = np.zeros(Etot, dtype=np.int32)
    idxA_pad[:E] = idxA_full
    idxB_pad = np.zeros(Etot, dtype=np.int32)
    idxB_pad[:E] = idxB_full

    in_maps = []
    for c in range(n_cores):
        sl = slice(c * Epc, (c + 1) * Epc)
        in_maps.append({
            "zTu": zTu, "zTm": zTm, "w1ut": w1ut, "w1mt": w1mt,
            "b1rep": b1rep, "b2rep": b2rep,
            "idxA": idxA_pad[sl].reshape(P, C),
            "idxB": idxB_pad[sl].reshape(P, C),
        })
    return in_maps, dict(C=C, NA=NA, NB=NB, Hp=Hp, E=E)



# ---------------------------------------------------------------------------
# v6: edges sorted by movie chunk; B-side expanded on PE via on-device one-hot
# (K=1 broadcast matmul + is_equal), eliminating all B-side gathers. A-side
# keeps single-column indirect gathers. Opt-in via EDGE_V6=1.
# ---------------------------------------------------------------------------

CPB = 2  # movie chunks per loop body


def _build_nc_v6(NA, NB, Hp, cap, repeat=1, repeat_pre=None, repeat_gather=None):
    f32 = mybir.dt.float32
    i32 = mybir.dt.int32
    tdt = mybir.dt.bfloat16
    MCH = NB // P
    SC = CPB * cap                 # slot-cols per body
    SLOTC = MCH * cap
    BODIES = MCH // CPB
    nc = bacc.Bacc(None, target_bir_lowering=False)
    SP = mybir.EngineType.SP

    zTu = nc.dram_tensor("zTu", [P, NA], tdt, kind="ExternalInput")
    zTm = nc.dram_tensor("zTm", [P, NB], tdt, kind="ExternalInput")
    w1ut = nc.dram_tensor("w1ut", [P, H], tdt, kind="ExternalInput")
    w1mt = nc.dram_tensor("w1mt", [P, H], tdt, kind="ExternalInput")
    b1rep = nc.dram_tensor("b1rep", [P, H], f32, kind="ExternalInput")
    b2rep = nc.dram_tensor("b2rep", [P, 1], f32, kind="ExternalInput")
    idxA = nc.dram_tensor("idxA", [P, SLOTC], i32, kind="ExternalInput")
    midsp = nc.dram_tensor("midsp", [1, BODIES * SC * P], tdt, kind="ExternalInput")
    iotas = nc.dram_tensor("iotas", [P, CPB], f32, kind="ExternalInput")
    out_d = nc.dram_tensor("out", [P, SLOTC], f32, kind="ExternalOutput")

    tabA = nc.dram_tensor("tabA", [NA, H], tdt)
    tabA_v = tabA[:].rearrange("(p m) d -> p (m d)", p=P)
    rp = repeat_pre if repeat_pre is not None else repeat
    rg = repeat_gather if repeat_gather is not None else repeat

    with tile.TileContext(nc) as tc:
        with (
            tc.tile_pool(name="const", bufs=1) as cpool,
            tc.tile_pool(name="work", bufs=4) as wpool,
            tc.tile_pool(name="psum", bufs=2, space="PSUM") as ppool,
            tc.tile_pool(name="psumb", bufs=2, space="PSUM") as pbpool,
        ):
            w1ut_t = cpool.tile([P, H], tdt)
            w1mt_t = cpool.tile([P, H], tdt)
            b1rep_t = cpool.tile([P, H], f32)
            b2rep_t = cpool.tile([P, 1], f32)
            zbias_t = cpool.tile([P, 1], f32)
            iotas_t = cpool.tile([P, CPB], f32)
            ones_t = cpool.tile([P, P], tdt)
            nc.sync.dma_start(out=w1ut_t[:], in_=w1ut[:])
            nc.sync.dma_start(out=w1mt_t[:], in_=w1mt[:])
            nc.sync.dma_start(out=b1rep_t[:], in_=b1rep[:])
            nc.sync.dma_start(out=b2rep_t[:], in_=b2rep[:])
            nc.sync.dma_start(out=iotas_t[:], in_=iotas[:])
            nc.vector.memset(zbias_t[:], 0.0)
            nc.vector.memset(ones_t[:], 1.0)

            for _r in range(rp):
                with tc.For_i(0, NA, ZBODY) as iv:
                    zstage = wpool.tile([P, ZBODY], tdt, tag="zstage")
                    nc.sync.dma_start(out=zstage[:], in_=zTu[:, bass.ds(iv, ZBODY)])
                    astage = wpool.tile([P, ZBODY], tdt, tag="astage")
                    for k in range(ZBODY // P):
                        ps = ppool.tile([P, H], f32, tag="ps")
                        nc.tensor.matmul(out=ps[:], lhsT=zstage[:, k * P:(k + 1) * P],
                                         rhs=w1ut_t[:], start=True, stop=True)
                        nc.vector.tensor_add(out=astage[:, k * H:(k + 1) * H],
                                             in0=ps[:], in1=b1rep_t[:])
                    nc.sync.dma_start(out=tabA_v[:, bass.ds(iv, ZBODY)], in_=astage[:])

            for _r in range(rg):
                with tc.For_i(0, MCH, CPB) as iv:
                    r_z = nc.alloc_registers(engines=[SP])
                    nc.regs_alu(r_z, iv, P, mybir.AluOpType.mult)
                    sv_z = nc.snap(r_z, donate=True)           # iv*128 (zTm cols)
                    r_i = nc.alloc_registers(engines=[SP])
                    nc.regs_alu(r_i, iv, cap, mybir.AluOpType.mult)
                    sv_i = nc.snap(r_i, donate=True)           # iv*cap (idx cols)
                    r_m = nc.alloc_registers(engines=[SP])
                    nc.regs_alu(r_m, iv, cap * P, mybir.AluOpType.mult)
                    sv_m = nc.snap(r_m, donate=True)           # (iv/CPB)*SC*128 (midsp cols)
                    r_o = nc.alloc_registers(engines=[SP])
                    nc.regs_alu(r_o, iv, cap, mybir.AluOpType.mult)
                    sv_o = nc.snap(r_o, donate=True)           # iv*cap (out cols)

                    zmst = wpool.tile([P, CPB * P], tdt, tag="zmst")
                    nc.sync.dma_start(out=zmst[:], in_=zTm[:, bass.ds(sv_z, CPB * P)])
                    ixst = wpool.tile([P, SC], i32, tag="ixst")
                    nc.sync.dma_start(out=ixst[:], in_=idxA[:, bass.ds(sv_i, SC)])
                    m0 = wpool.tile([P, SC * P], tdt, tag="m0")
                    nc.sync.dma_start(out=m0[0:1, :], in_=midsp[0:1, bass.ds(sv_m, SC * P)])

                    bch = []
                    for c in range(CPB):
                        psB = ppool.tile([P, H], f32, tag="psB")
                        nc.tensor.matmul(out=psB[:], lhsT=zmst[:, c * P:(c + 1) * P],
                                         rhs=w1mt_t[:], start=True, stop=True)
                        bc = wpool.tile([P, H], tdt, tag=f"bch{c}")
                        nc.scalar.copy(out=bc[:], in_=psB[:])
                        bch.append(bc)

                    ct = wpool.tile([P, SC * H], tdt, tag="ct")
                    cta = wpool.tile([P, SC * H], tdt, tag="cta")
                    for s in range(SC):
                        psbc = pbpool.tile([P, P], f32, tag="psbc")
                        nc.tensor.matmul(out=psbc[:], lhsT=ones_t[0:1, :],
                                         rhs=m0[0:1, s * P:(s + 1) * P], start=True, stop=True)
                        pse = pbpool.tile([P, H], f32, tag="pse")
                        for c in range(CPB):
                            S_sb = wpool.tile([P, P], tdt, tag=f"S{c}")
                            nc.vector.tensor_scalar(
                                out=S_sb[:], in0=psbc[:], scalar1=iotas_t[:, c:c + 1],
                                scalar2=None, op0=mybir.AluOpType.is_equal)
                            nc.tensor.matmul(out=pse[:], lhsT=S_sb[:], rhs=bch[c][:],
                                             start=(c == 0), stop=(c == CPB - 1))
                        nc.gpsimd.indirect_dma_start(
                            out=cta[:, s * H:(s + 1) * H], out_offset=None, in_=tabA[:],
                            in_offset=bass.IndirectOffsetOnAxis(ap=ixst[:, s:s + 1], axis=0))
                        pseb = wpool.tile([P, H], tdt, tag="pseb")
                        nc.scalar.copy(out=pseb[:], in_=pse[:])
                        nc.vector.tensor_add(out=ct[:, s * H:(s + 1) * H],
                                             in0=cta[:, s * H:(s + 1) * H], in1=pseb[:])

                    nc.scalar.activation(out=ct[:], in_=ct[:],
                                         func=mybir.ActivationFunctionType.Relu,
                                         bias=zbias_t[:, 0:1], scale=1.0)
                    cc = ct[:].rearrange("p (g h) -> p g h", h=H)
                    pos = wpool.tile([P, SC], f32, tag="pos")
                    nc.vector.tensor_reduce(out=pos[:], in_=cc[:, :, 0:Hp],
                                            axis=mybir.AxisListType.X, op=mybir.AluOpType.add)
                    neg = wpool.tile([P, SC], f32, tag="neg")
                    nc.vector.tensor_reduce(out=neg[:], in_=cc[:, :, Hp:H],
                                            axis=mybir.AxisListType.X, op=mybir.AluOpType.add)
                    lg = wpool.tile([P, SC], f32, tag="lg")
                    nc.vector.tensor_sub(out=lg[:], in0=pos[:], in1=neg[:])
                    sg = wpool.tile([P, SC], f32, tag="sg")
                    nc.scalar.activation(out=sg[:], in_=lg[:],
                                         func=mybir.ActivationFunctionType.Sigmoid,
                                         bias=b2rep_t[:, 0:1], scale=1.0)
                    nc.scalar.mul(out=sg[:], in_=sg[:], mul=5.0)
                    nc.sync.dma_start(out=out_d[:, bass.ds(sv_o, SC)], in_=sg[:])
    nc.finalize()
    return nc


def _prepare_v6(z_user, z_movie, edge_index, W1, b1, W2, b2, n_cores=N_CORES):
    z_user = np.asarray(z_user, dtype=np.float32)
    z_movie = np.asarray(z_movie, dtype=np.float32)
    edge_index = np.asarray(edge_index)
    W1 = np.asarray(W1, dtype=np.float32)
    b1 = np.asarray(b1, dtype=np.float32)
    W2 = np.asarray(W2, dtype=np.float32)
    b2 = np.asarray(b2, dtype=np.float32)
    tnp = ml_dtypes.bfloat16

    E = edge_index.shape[1]
    rows = edge_index[0].astype(np.int64)
    cols = edge_index[1].astype(np.int64)
    NAr = int(rows.max()) + 1 if E else 1
    NBr = z_movie.shape[0]
    NA = _pad_cols(max(NAr, ZBODY), ZBODY)
    NB = _pad_cols(max(NBr, ZBODY), ZBODY)
    MCH = NB // P
    assert MCH % CPB == 0

    w2 = W2.reshape(-1)
    perm = np.argsort(w2 < 0, kind="stable")
    Hp = int((w2 >= 0).sum())
    assert 0 < Hp < H
    W1p = W1[perm]
    b1p = b1[perm]
    scale = np.abs(w2[perm])
    w1ut = np.ascontiguousarray((W1p[:, :H] * scale[:, None]).T).astype(tnp)
    w1mt = np.ascontiguousarray((W1p[:, H:] * scale[:, None]).T).astype(tnp)
    b1rep = np.tile(b1p * scale, (P, 1)).astype(np.float32)
    b2rep = np.full((P, 1), float(b2.reshape(-1)[0]), dtype=np.float32)

    zTu = np.zeros((P, NA), dtype=tnp)
    zTu[:, :NAr] = z_user[:NAr].T.astype(tnp)
    zTm = np.zeros((P, NB), dtype=tnp)
    zTm[:, :NBr] = z_movie.T.astype(tnp)
    mA = NA // P

    Epc = -(-E // n_cores)
    core_data = []
    cap = 1
    for c in range(n_cores):
        sl = slice(c * Epc, min((c + 1) * Epc, E))
        rc, cc = rows[sl], cols[sl]
        order = np.argsort(cc, kind="stable")
        rs, cs = rc[order], cc[order]
        cnt = np.bincount(cs // P, minlength=MCH)
        cap = max(cap, int(-(-cnt.max() // P)))
        core_data.append((sl, order, rs, cs, cnt))

    SLOTC = MCH * cap
    BODIES = MCH // CPB
    SC = CPB * cap
    iotas = (np.arange(P)[:, None] + P * np.arange(CPB)[None, :]).astype(np.float32)

    in_maps, backmaps = [], []
    for (sl, order, rs, cs, cnt) in core_data:
        m_e = cs // P
        start = np.zeros(MCH + 1, dtype=np.int64)
        np.cumsum(cnt, out=start[1:])
        j_e = np.arange(len(cs)) - start[m_e]
        t_e = j_e // P
        p_e = j_e % P
        slotcol = m_e * cap + t_e
        idxA_blob = np.zeros((P, SLOTC), dtype=np.int32)
        idxA_blob[p_e, slotcol] = ((rs % P) * mA + rs // P).astype(np.int32)
        mids = np.full((P, SLOTC), 999.0, dtype=np.float32)
        mids[p_e, slotcol] = (cs % P) + P * (m_e % CPB)
        # midsp[0, body*SC*128 + s*128 + q] = mids[q, body*SC + s]
        m3 = mids.reshape(P, BODIES, SC)          # [q, body, s]
        midsp = np.ascontiguousarray(
            np.transpose(m3, (1, 2, 0)).reshape(1, BODIES * SC * P)).astype(tnp)
        in_maps.append({
            "zTu": zTu, "zTm": zTm, "w1ut": w1ut, "w1mt": w1mt,
            "b1rep": b1rep, "b2rep": b2rep, "iotas": iotas,
            "idxA": idxA_blob, "midsp": midsp,
        })
        backmaps.append((sl, order, p_e, slotcol))
    return in_maps, dict(NA=NA, NB=NB, Hp=Hp, cap=cap, E=E,
                         SLOTC=SLOTC, backmaps=backmaps)


def _unpack_v6(res, meta):
    out = np.empty(meta["E"], dtype=np.float32)
    for c, (sl, order, p_e, slotcol) in enumerate(meta["backmaps"]):
        vals = res.results[c]["out"]
        sorted_vals = vals[p_e, slotcol]
        seg = np.empty(len(order), dtype=np.float32)
        seg[order] = sorted_vals
        out[sl] = seg
    return out


def kernel(z_user, z_movie, edge_index, W1, b1, W2, b2):
    import os
    if os.environ.get("EDGE_V6") == "1":
        in_maps, meta = _prepare_v6(z_user, z_movie, edge_index, W1, b1, W2, b2)
        nc = _build_nc_v6(meta["NA"], meta["NB"], meta["Hp"], meta["cap"])
        res = run_bass_kernel_spmd(nc, in_maps, core_ids=list(range(N_CORES)))
        out = _unpack_v6(res, meta)
        _LAST_STATS.update(exec_time_ns=res.exec_time_ns, nc=nc,
                           in_maps=in_maps, meta=meta, version="v6")
        return out
    in_maps, meta = _prepare(z_user, z_movie, edge_index, W1, b1, W2, b2)
    nc = _build_nc(meta["C"], meta["NA"], meta["NB"], meta["Hp"])
    res = run_bass_kernel_spmd(nc, in_maps, core_ids=list(range(N_CORES)))
    out = np.concatenate([res.results[c]["out"].reshape(-1) for c in range(N_CORES)])
    _LAST_STATS.update(exec_time_ns=res.exec_time_ns, nc=nc,
                       in_maps=in_maps, meta=meta, version="v3")
    return out[:meta["E"]].astype(np.float32)
